# revision 15
# baseline (speedup 1.0000x reference)
"""Trainium2 Bass kernel for nn_Attention_2010044694916.

Dense transformer attention block:
  x:(128,245,768) -> qkv proj -> 12-head attention (+RPE bias, softmax)
  -> out proj (+bias) -> y:(128,245,768)

Strategy: pure data-parallel over batch across 8 NeuronCores (16 items
per core, processed in pairs). All layout transposes are done on the
host so the device program contains zero PE transposes:

  - x is pre-transposed per item to xT [768, 256pad] (bf16, zero-padded)
  - q/k are computed in transposed layout qkT[f, t] = (Wqk @ xT)
    (lhsT = Wqk^T host-pretransposed); v in normal layout v[t, f]
  - scores are computed directly transposed: ST[j,i] = kT^T.T... i.e.
    matmul(lhsT=kT_h, rhs=qT_h) -> [j, i] so that the attn@v matmul
    needs no transpose at all
  - softmax is max-free (scores are O(1) here), bias folded in as
    E = exp(ST) * exp(biasT) with exp(biasT) precomputed on host
  - row sums l_i come from ones-pattern matmuls; 1/l is broadcast
    across partitions with gpsimd partition_broadcast
  - projection consumes outT directly; proj bias fused into the PSUM
    drain on ScalarE; output written transposed, host undoes it.
"""

import functools

import numpy as np

B, N, D, H, DH = 128, 245, 768, 12, 64
NP = 256  # padded token count per item
NCORES = 8
BL = B // NCORES  # items per core
PAIRS = BL // 2
SCALE = DH ** -0.5
JSZ = (128, N - 128)  # j-chunk sizes (128, 117)
STAGE = 99  # debug: truncate device program after this pipeline stage


# ----------------------------------------------------------------- host prep

def _bf16():
    import ml_dtypes

    return ml_dtypes.bfloat16


def _prep_weights(qkv_w, proj_w, proj_b, bias_table, rel_index):
    """Host-side preprocessing of all per-core-replicated tensors."""
    bf16 = _bf16()
    qkv_w = np.asarray(qkv_w, np.float32)
    proj_w = np.asarray(proj_w, np.float32)
    proj_b = np.asarray(proj_b, np.float32)
    bias_table = np.asarray(bias_table, np.float32)
    rel_index = np.asarray(rel_index)

    wqk = np.concatenate([qkv_w[:D] * SCALE, qkv_w[D : 2 * D]], axis=0)  # (1536,768)
    # lhsT tiles for qk projection: wqkT[c,m] with c on partitions
    wqk_h = np.ascontiguousarray(
        wqk.T.reshape(6, 128, 2 * D).transpose(1, 0, 2)
    ).astype(bf16)  # (128, 6, 1536)
    wv_h = np.ascontiguousarray(
        qkv_w[2 * D :].T.reshape(6, 128, D).transpose(1, 0, 2)
    ).astype(bf16)  # (128, 6, 768)
    pw_h = np.ascontiguousarray(
        proj_w.T.reshape(6, 128, D).transpose(1, 0, 2)
    ).astype(bf16)  # (128, 6, 768)
    pb_h = np.ascontiguousarray(proj_b.reshape(6, 128).T).astype(np.float32)  # (128,6)

    # exp of transposed bias, padded-i cols = 1.0, layout [jc, j, h*256+i]
    bias_full = bias_table[:, rel_index]  # (12, 245, 245) [h, i, j]
    biasT = bias_full.transpose(0, 2, 1)  # [h, j, i]
    ebt = np.ones((2, 128, H, NP), np.float32)
    ebt[0, :128, :, :N] = np.exp(biasT[:, 0:128, :]).transpose(1, 0, 2)
    ebt[1, : JSZ[1], :, :N] = np.exp(biasT[:, 128:N, :]).transpose(1, 0, 2)
    ebt_h = np.ascontiguousarray(ebt.reshape(2, 128, H * NP)).astype(bf16)

    # one-hot column patterns for the l (row-sum) matmuls
    lhot_h = np.zeros((128, H * H), np.float32)
    for h in range(H):
        lhot_h[:, h * H + h] = 1.0
    lhot_h = lhot_h.astype(bf16)

    return dict(wqk=wqk_h, wv=wv_h, pw=pw_h, pb=pb_h, ebt=ebt_h, lhot=lhot_h)


def _prep_x_core(xc):
    """xc (BL,245,768) f32 -> xt (PAIRS,128,6,512) bf16, zero-padded."""
    bf16 = _bf16()
    xp = np.zeros((BL, D, NP), np.float32)
    xp[:, :, :N] = np.asarray(xc, np.float32).transpose(0, 2, 1)
    xt = (
        xp.reshape(PAIRS, 2, 6, 128, NP)
        .transpose(0, 3, 2, 1, 4)
        .reshape(PAIRS, 128, 6, 2 * NP)
    )
    return np.ascontiguousarray(xt).astype(bf16)


def _decode_y_core(yt):
    """yt (PAIRS,128,6,512) f32 -> y (BL,245,768) f32."""
    arr = (
        np.asarray(yt, np.float32)
        .reshape(PAIRS, 128, 6, 2, NP)
        .transpose(0, 3, 4, 2, 1)
        .reshape(BL // 2 * 2 and PAIRS * 2, NP, D)[:, :N, :]
    )
    return np.ascontiguousarray(arr)


# ------------------------------------------------------------- bass program

def _patch_tile_drain():
    """This walrus build only accepts one sync-wait on a Drain instruction;
    spread the Tile tail-drain waits over several drains."""
    import concourse.tile as tile
    from concourse import mybir
    from concourse.vector_clock import ScopedClock

    if getattr(tile.TileContext, "_drain_patched", False):
        return

    def _drain_and_barrier(self, tick_clock, wait_clock):
        drain_inst = self.nc.sync.drain()
        wait_clock.add_sem_waits(
            drain_inst.ins, ScopedClock({None: tick_clock.global_clock})
        )
        si = drain_inst.ins.sync_info
        waits = list(si.on_wait)
        if len(waits) > 1:
            drain_inst.ins.sync_info = mybir.SyncInfo(
                on_wait=waits[:1], on_update=list(si.on_update)
            )
            for i in range(1, len(waits)):
                extra = self.nc.sync.drain()
                extra.ins.sync_info = mybir.SyncInfo(
                    on_wait=waits[i : i + 1], on_update=[]
                )
        self.nc.all_engine_barrier()
        assert self.sems is not None
        popped = self.nc._tile_sem_poison_stack.pop()
        assert popped is self._sem_poison
        self.nc.clear_and_free_semaphores(list(self.sems.allocated().values()))
        self.nc.all_engine_barrier()

    tile.TileContext._drain_and_barrier = _drain_and_barrier
    tile.TileContext._drain_patched = True


def _build_bass():
    import concourse.bass as bass
    import concourse.tile as tile
    from concourse import bacc
    from concourse import mybir

    _patch_tile_drain()

    bf = mybir.dt.bfloat16
    f32 = mybir.dt.float32
    Exp = mybir.ActivationFunctionType.Exp
    Ident = mybir.ActivationFunctionType.Identity

    nc = bacc.Bacc()
    xt_d = nc.dram_tensor("xt", [PAIRS, 128, 6, 2 * NP], bf, kind="ExternalInput")
    wqk_d = nc.dram_tensor("wqk", [128, 6, 2 * D], bf, kind="ExternalInput")
    wv_d = nc.dram_tensor("wv", [128, 6, D], bf, kind="ExternalInput")
    pw_d = nc.dram_tensor("pw", [128, 6, D], bf, kind="ExternalInput")
    pb_d = nc.dram_tensor("pb", [128, 6], f32, kind="ExternalInput")
    ebt_d = nc.dram_tensor("ebt", [2, 128, H * NP], bf, kind="ExternalInput")
    lhot_d = nc.dram_tensor("lhot", [128, H * H], bf, kind="ExternalInput")
    yt_d = nc.dram_tensor("yt", [PAIRS, 128, 6, 2 * NP], f32, kind="ExternalOutput")

    with tile.TileContext(nc) as tc:
        with (
            tc.tile_pool(name="const", bufs=1) as constp,
            tc.tile_pool(name="px", bufs=2) as px,
            tc.tile_pool(name="pqk", bufs=2) as pqk,
            tc.tile_pool(name="pv", bufs=2) as pv,
            tc.tile_pool(name="per", bufs=3) as per,
            tc.tile_pool(name="pet", bufs=4) as pet,
            tc.tile_pool(name="prc", bufs=2) as prc,
            tc.tile_pool(name="prb", bufs=2) as prb,
            tc.tile_pool(name="pot", bufs=2) as pot,
            tc.tile_pool(name="py", bufs=2) as pysb,
            tc.tile_pool(name="pdram", bufs=2, space="DRAM") as pdram,
            tc.tile_pool(name="ppq", bufs=2, space="PSUM") as ppq,
            tc.tile_pool(name="pst", bufs=3, space="PSUM") as pst,
            tc.tile_pool(name="ppo", bufs=3, space="PSUM") as ppo,
        ):
            wqk_sb = constp.tile([128, 6, 2 * D], bf)
            nc.sync.dma_start(wqk_sb[:], wqk_d[:])
            wv_sb = constp.tile([128, 6, D], bf)
            nc.sync.dma_start(wv_sb[:], wv_d[:])
            pw_sb = constp.tile([128, 6, D], bf)
            nc.sync.dma_start(pw_sb[:], pw_d[:])
            pb_sb = constp.tile([128, 6], f32)
            nc.sync.dma_start(pb_sb[:], pb_d[:])
            ebt_sb = [constp.tile([128, H * NP], bf, name=f"ebt{j}") for j in range(2)]
            for j in range(2):
                nc.sync.dma_start(ebt_sb[j][:], ebt_d[j])
            lhot_sb = constp.tile([128, H * H], bf)
            nc.sync.dma_start(lhot_sb[:], lhot_d[:])

            for p in range(PAIRS):
                xt = px.tile([128, 6, 2 * NP], bf, tag="xt")
                nc.sync.dma_start(xt[:], xt_d[p])

                # ---- qk projection, transposed output [feat, tok]
                qk_sb = pqk.tile([128, H, 2 * NP], bf, tag="qk")
                for m in range(12):
                    ps = ppq.tile([128, 2 * NP], f32, tag="qv")
                    for kc in range(6):
                        nc.tensor.matmul(
                            ps[:],
                            lhsT=wqk_sb[:, kc, m * 128 : (m + 1) * 128],
                            rhs=xt[:, kc, :],
                            start=(kc == 0),
                            stop=(kc == 5),
                        )
                    nc.vector.tensor_copy(qk_sb[:, m, :], ps[:])

                if STAGE < 2:
                    y_sb = pysb.tile([128, 6, 2 * NP], f32, tag="y")
                    nc.vector.tensor_copy(y_sb[:], qk_sb[:, :6, :])
                    nc.sync.dma_start(yt_d[p], y_sb[:])
                    continue

                # ---- v projection, normal layout [tok, feat]
                v_sb = pv.tile([128, 4, D], bf, tag="v")
                for mc in range(4):
                    for nh in range(2):
                        ps = ppq.tile([128, 2 * NP], f32, tag="qv")
                        psl_ = ps[:, : D // 2]
                        for kc in range(6):
                            nc.tensor.matmul(
                                psl_,
                                lhsT=xt[:, kc, mc * 128 : (mc + 1) * 128],
                                rhs=wv_sb[:, kc, nh * (D // 2) : (nh + 1) * (D // 2)],
                                start=(kc == 0),
                                stop=(kc == 5),
                            )
                        nc.vector.tensor_copy(
                            v_sb[:, mc, nh * (D // 2) : (nh + 1) * (D // 2)], psl_
                        )

                if STAGE < 3:
                    y_sb = pysb.tile([128, 6, 2 * NP], f32, tag="y")
                    nc.vector.tensor_copy(y_sb[:, :4, :], v_sb[:, :, :512])
                    nc.sync.dma_start(yt_d[p], y_sb[:])
                    continue

                ot = pot.tile([128, 6, 2 * NP], bf, tag="ot")

                for it in range(2):
                    tb = it * NP
                    # ---- scores (transposed) + exp + bias-multiply
                    et = [
                        pet.tile([128, H * NP], bf, tag="et", name=f"et{jc}")
                        for jc in range(2)
                    ]
                    for jc in range(2):
                        jsz = JSZ[jc]
                        jst = tb + jc * 128
                        for h in range(H):
                            ps = pst.tile([128, NP], f32, tag="st")
                            fc, ko = h // 2, (h % 2) * 64
                            nc.tensor.matmul(
                                ps[0:jsz, :],
                                lhsT=qk_sb[ko : ko + 64, 6 + fc, jst : jst + jsz],
                                rhs=qk_sb[ko : ko + 64, fc, tb : tb + NP],
                                start=True,
                                stop=True,
                            )
                            er = per.tile([128, NP], bf, tag="er")
                            nc.scalar.activation(er[0:jsz, :], ps[0:jsz, :], func=Exp)
                            nc.vector.tensor_mul(
                                et[jc][0:jsz, h * NP : (h + 1) * NP],
                                er[0:jsz, :],
                                ebt_sb[jc][0:jsz, h * NP : (h + 1) * NP],
                            )

                    if STAGE < 4:
                        nc.vector.tensor_copy(ot[:, 0, tb : tb + NP], et[0][:, :NP])
                        continue

                    # ---- row sums l[h, i] via one-hot matmuls
                    psl = ppo.tile([H, NP], f32, tag="ol", name="psl")
                    nmm = 0
                    for jc in range(2):
                        jsz = JSZ[jc]
                        for h in range(H):
                            nc.tensor.matmul(
                                psl[:],
                                lhsT=lhot_sb[0:jsz, h * H : (h + 1) * H],
                                rhs=et[jc][0:jsz, h * NP : (h + 1) * NP],
                                start=(nmm == 0),
                                stop=(nmm == 2 * H - 1),
                            )
                            nmm += 1
                    rcp = prc.tile([H, NP], f32, tag="rcp")
                    nc.vector.reciprocal(rcp[:], psl[:])
                    rdr = pdram.tile([H, NP], f32, tag="rd")
                    nc.sync.dma_start(rdr[:], rcp[:])
                    rb = prb.tile([128, 6, NP], f32, tag="rb")
                    for hh in range(2):
                        src = bass.AP(
                            tensor=rdr.tensor,
                            offset=rdr.offset + hh * NP,
                            ap=[[0, 64], [2 * NP, 6], [1, NP]],
                        )
                        nc.sync.dma_start(rb[hh * 64 : (hh + 1) * 64, :, :], src)

                    if STAGE < 5:
                        for g in range(6):
                            nc.vector.tensor_copy(ot[:, g, tb : tb + NP], rb[:, g, :])
                        continue

                    # ---- attn @ v (outT layout) + 1/l normalize into ot
                    for g in range(6):
                        pso = ppo.tile([128, NP], f32, tag="ol", name="pso")
                        for hh in range(2):
                            h = 2 * g + hh
                            for jc in range(2):
                                jsz = JSZ[jc]
                                nc.tensor.matmul(
                                    pso[hh * 64 : (hh + 1) * 64, :],
                                    lhsT=v_sb[0:jsz, it * 2 + jc, h * 64 : (h + 1) * 64],
                                    rhs=et[jc][0:jsz, h * NP : (h + 1) * NP],
                                    start=(jc == 0),
                                    stop=(jc == 1),
                                    tile_position=(0, hh * 64),
                                )
                        nc.vector.tensor_mul(
                            ot[:, g, tb : tb + NP], pso[:], rb[:, g, :]
                        )

                if STAGE < 6:
                    y_sb = pysb.tile([128, 6, 2 * NP], f32, tag="y")
                    nc.vector.tensor_copy(y_sb[:], ot[:])
                    nc.sync.dma_start(yt_d[p], y_sb[:])
                    continue

                # ---- output projection + bias, write transposed y
                y_sb = pysb.tile([128, 6, 2 * NP], f32, tag="y")
                for nn_ in range(6):
                    psy = ppo.tile([128, 2 * NP], f32, tag="ol", name="psy")
                    for cc in range(6):
                        nc.tensor.matmul(
                            psy[:],
                            lhsT=pw_sb[:, cc, nn_ * 128 : (nn_ + 1) * 128],
                            rhs=ot[:, cc, :],
                            start=(cc == 0),
                            stop=(cc == 5),
                        )
                    nc.scalar.activation(
                        y_sb[:, nn_, :],
                        psy[:],
                        func=Ident,
                        bias=pb_sb[:, nn_ : nn_ + 1],
                        scale=1.0,
                    )
                nc.sync.dma_start(yt_d[p], y_sb[:])

    nc.compile()
    return nc


# ----------------------------------------------------------------- execution

@functools.cache
def _get_runner():
    """Build the bass program once and return a cached jitted executor.

    Mirrors concourse.bass2jax.run_bass_via_pjrt but caches the jit so
    repeated kernel() calls (and timing loops) do not recompile.
    """
    import jax
    import jax.numpy as jnp
    from jax.sharding import Mesh, PartitionSpec
    from jax.experimental.shard_map import shard_map

    from concourse import mybir
    from concourse import bass2jax

    bass2jax.install_neuronx_cc_hook()
    nc = _build_bass()

    partition_name = (
        nc.partition_id_tensor.name if nc.partition_id_tensor is not None else None
    )
    in_names, out_names, out_avals = [], [], []
    for alloc in nc.m.functions[0].allocations:
        if not isinstance(alloc, mybir.MemoryLocationSet):
            continue
        name = alloc.memorylocations[0].name
        if alloc.kind == "ExternalInput":
            if name != partition_name:
                in_names.append(name)
        elif alloc.kind == "ExternalOutput":
            out_names.append(name)
            out_avals.append(
                jax.core.ShapedArray(
                    tuple(alloc.tensor_shape), mybir.dt.np(alloc.dtype)
                )
            )
    n_params = len(in_names)
    all_in_names = tuple(in_names + out_names)
    if partition_name is not None:
        all_in_names = all_in_names + (partition_name,)

    def _body(*args):
        operands = list(args)
        if partition_name is not None:
            operands.append(bass2jax.partition_id_tensor())
        outs = bass2jax._bass_exec_p.bind(
            *operands,
            out_avals=tuple(out_avals),
            in_names=all_in_names,
            out_names=tuple(out_names),
            lowering_input_output_aliases=(),
            sim_require_finite=True,
            sim_require_nnan=True,
            nc=nc,
        )
        return tuple(outs)

    devices = jax.devices()[:NCORES]
    mesh = Mesh(np.asarray(devices), ("core",))
    n_outs = len(out_names)
    donate = tuple(range(n_params, n_params + n_outs))
    sharded = jax.jit(
        shard_map(
            _body,
            mesh=mesh,
            in_specs=(PartitionSpec("core"),) * (n_params + n_outs),
            out_specs=(PartitionSpec("core"),) * n_outs,
            check_rep=False,
        ),
        donate_argnums=donate,
        keep_unused=True,
    )
    return sharded, in_names, out_names, out_avals


def _run_device(per_core_inputs):
    """per_core_inputs: list (len 8) of dicts name->np array. Returns list of
    dicts name->np array (outputs)."""
    sharded, in_names, out_names, out_avals = _get_runner()
    concat_in = [
        np.concatenate([per_core_inputs[c][nm] for c in range(NCORES)], axis=0)
        for nm in in_names
    ]
    concat_zeros = [
        np.zeros((NCORES * a.shape[0], *a.shape[1:]), a.dtype) for a in out_avals
    ]
    out_arrs = sharded(*concat_in, *concat_zeros)
    res = []
    for c in range(NCORES):
        res.append(
            {
                nm: np.asarray(out_arrs[i]).reshape(NCORES, *out_avals[i].shape)[c]
                for i, nm in enumerate(out_names)
            }
        )
    return res


def kernel(x, qkv_w, proj_w, proj_b, bias_table, rel_index):
    x = np.asarray(x, np.float32)
    w = _prep_weights(qkv_w, proj_w, proj_b, bias_table, rel_index)
    per_core = []
    for c in range(NCORES):
        m = dict(w)
        m["xt"] = _prep_x_core(x[c * BL : (c + 1) * BL])
        per_core.append(m)
    res = _run_device(per_core)
    y = np.concatenate([_decode_y_core(res[c]["yt"]) for c in range(NCORES)], axis=0)
    return y.astype(np.float32)


# ------------------------------------------------- numpy emulation (debug)

def _numpy_sim(x, qkv_w, proj_w, proj_b, bias_table, rel_index, exact=False):
    """Bit-layout-faithful numpy emulation of the device program."""
    bf16 = _bf16()
    cast = (lambda a: a.astype(np.float32)) if exact else (
        lambda a: a.astype(bf16).astype(np.float32)
    )
    w = _prep_weights(qkv_w, proj_w, proj_b, bias_table, rel_index)
    wqk = w["wqk"].astype(np.float32)  # (128, 6, 1536)
    wv = w["wv"].astype(np.float32)
    pw = w["pw"].astype(np.float32)
    pb = w["pb"].astype(np.float32)
    ebt = w["ebt"].astype(np.float32).reshape(2, 128, H, NP)
    x = np.asarray(x, np.float32)
    y_all = np.zeros((B, N, D), np.float32)
    for c in range(NCORES):
        xt = _prep_x_core(x[c * BL : (c + 1) * BL]).astype(np.float32)
        yt = np.zeros((PAIRS, 128, 6, 2 * NP), np.float32)
        for p in range(PAIRS):
            xtp = xt[p]  # (128, 6, 512)
            qk = np.zeros((128, H, 2 * NP), np.float32)
            for m in range(12):
                acc = np.zeros((128, 2 * NP), np.float32)
                for kc in range(6):
                    acc += wqk[:, kc, m * 128 : (m + 1) * 128].T @ xtp[:, kc, :]
                qk[:, m, :] = cast(acc)
            v = np.zeros((128, 4, D), np.float32)
            for mc in range(4):
                acc = np.zeros((128, D), np.float32)
                for kc in range(6):
                    acc += xtp[:, kc, mc * 128 : (mc + 1) * 128].T @ wv[:, kc, :]
                v[:, mc, :] = cast(acc)
            ot = np.zeros((128, 6, 2 * NP), np.float32)
            for it in range(2):
                tb = it * NP
                et = [np.zeros((128, H, NP), np.float32) for _ in range(2)]
                for jc in range(2):
                    jsz = JSZ[jc]
                    jst = tb + jc * 128
                    for h in range(H):
                        fc, ko = h // 2, (h % 2) * 64
                        st = (
                            qk[ko : ko + 64, 6 + fc, jst : jst + jsz].T
                            @ qk[ko : ko + 64, fc, tb : tb + NP]
                        )
                        er = cast(np.exp(st))
                        et[jc][0:jsz, h, :] = cast(er * ebt[jc][0:jsz, h, :])
                l = np.zeros((H, NP), np.float32)
                for jc in range(2):
                    l += et[jc][: JSZ[jc]].sum(axis=0)
                rcp = 1.0 / l
                for g in range(6):
                    for hh in range(2):
                        h = 2 * g + hh
                        acc = np.zeros((64, NP), np.float32)
                        for jc in range(2):
                            jsz = JSZ[jc]
                            acc += (
                                v[0:jsz, it * 2 + jc, h * 64 : (h + 1) * 64].T
                                @ et[jc][0:jsz, h, :]
                            )
                        ot[hh * 64 : (hh + 1) * 64, g, tb : tb + NP] = cast(
                            acc * rcp[h][None, :]
                        )
                for nn_ in range(6):
                    acc = np.zeros((128, 2 * NP), np.float32)
                    for cc in range(6):
                        acc += pw[:, cc, nn_ * 128 : (nn_ + 1) * 128].T @ ot[:, cc, :]
                    yt[p, :, nn_, :] = acc + pb[:, nn_ : nn_ + 1]
        y_all[c * BL : (c + 1) * BL] = _decode_y_core(yt)
    return y_all


# revision 21
# speedup vs baseline: 65.4199x; 65.4199x over previous
"""Trainium2 Bass kernel for nn_Attention_2010044694916.

Dense transformer attention block:
  x:(128,245,768) -> qkv proj -> 12-head attention (+RPE bias, softmax)
  -> out proj (+bias) -> y:(128,245,768)

Strategy: pure data-parallel over batch across 8 NeuronCores (16 items
per core, processed in pairs). All layout transposes are done on the
host so the device program contains zero PE transposes:

  - x is pre-transposed per item to xT [768, 256pad] (bf16, zero-padded)
  - q/k are computed in transposed layout qkT[f, t] = (Wqk @ xT)
    (lhsT = Wqk^T host-pretransposed); v in normal layout v[t, f]
  - scores are computed directly transposed: ST[j,i] = kT^T.T... i.e.
    matmul(lhsT=kT_h, rhs=qT_h) -> [j, i] so that the attn@v matmul
    needs no transpose at all
  - softmax is max-free (scores are O(1) here), bias folded in as
    E = exp(ST) * exp(biasT) with exp(biasT) precomputed on host
  - row sums l_i come from ones-pattern matmuls; 1/l is broadcast
    across partitions with gpsimd partition_broadcast
  - projection consumes outT directly; proj bias fused into the PSUM
    drain on ScalarE; output written transposed, host undoes it.
"""

import functools

import numpy as np

B, N, D, H, DH = 128, 245, 768, 12, 64
NP = 256  # padded token count per item
NCORES = 8
BL = B // NCORES  # items per core
PAIRS = BL // 2
SCALE = DH ** -0.5
JSZ = (128, N - 128)  # j-chunk sizes (128, 117)
STAGE = 99  # debug: truncate device program after this pipeline stage
REPS = 1  # debug: replicate whole workload inside the NEFF (timing)


# ----------------------------------------------------------------- host prep

def _bf16():
    import ml_dtypes

    return ml_dtypes.bfloat16


def _prep_weights(qkv_w, proj_w, proj_b, bias_table, rel_index):
    """Host-side preprocessing of all per-core-replicated tensors."""
    bf16 = _bf16()
    qkv_w = np.asarray(qkv_w, np.float32)
    proj_w = np.asarray(proj_w, np.float32)
    proj_b = np.asarray(proj_b, np.float32)
    bias_table = np.asarray(bias_table, np.float32)
    rel_index = np.asarray(rel_index)

    wqk = np.concatenate([qkv_w[:D] * SCALE, qkv_w[D : 2 * D]], axis=0)  # (1536,768)
    # lhsT tiles for qk projection: wqkT[c,m] with c on partitions
    wqk_h = np.ascontiguousarray(
        wqk.T.reshape(6, 128, 2 * D).transpose(1, 0, 2)
    ).astype(bf16)  # (128, 6, 1536)
    wv_h = np.ascontiguousarray(
        qkv_w[2 * D :].T.reshape(6, 128, D).transpose(1, 0, 2)
    ).astype(bf16)  # (128, 6, 768)
    pw_h = np.ascontiguousarray(
        proj_w.T.reshape(6, 128, D).transpose(1, 0, 2)
    ).astype(bf16)  # (128, 6, 768)
    pb_h = np.ascontiguousarray(proj_b.reshape(6, 128).T).astype(np.float32)  # (128,6)

    # exp of transposed bias, padded-i cols = 1.0, layout [jc, j, h*256+i]
    bias_full = bias_table[:, rel_index]  # (12, 245, 245) [h, i, j]
    biasT = bias_full.transpose(0, 2, 1)  # [h, j, i]
    ebt = np.ones((2, 128, H, NP), np.float32)
    ebt[0, :128, :, :N] = np.exp(biasT[:, 0:128, :]).transpose(1, 0, 2)
    ebt[1, : JSZ[1], :, :N] = np.exp(biasT[:, 128:N, :]).transpose(1, 0, 2)
    ebt_h = np.ascontiguousarray(ebt.reshape(2, 128, H * NP)).astype(bf16)

    # one-hot column patterns for the l (row-sum) matmuls
    lhot_h = np.zeros((128, H * H), np.float32)
    for h in range(H):
        lhot_h[:, h * H + h] = 1.0
    lhot_h = lhot_h.astype(bf16)

    return dict(wqk=wqk_h, wv=wv_h, pw=pw_h, pb=pb_h, ebt=ebt_h, lhot=lhot_h)


def _prep_x_core(xc):
    """xc (BL,245,768) f32 -> xt (PAIRS,128,6,512) bf16, zero-padded."""
    bf16 = _bf16()
    xp = np.zeros((BL, D, NP), np.float32)
    xp[:, :, :N] = np.asarray(xc, np.float32).transpose(0, 2, 1)
    xt = (
        xp.reshape(PAIRS, 2, 6, 128, NP)
        .transpose(0, 3, 2, 1, 4)
        .reshape(PAIRS, 128, 6, 2 * NP)
    )
    return np.ascontiguousarray(xt).astype(bf16)


def _decode_y_core(yt):
    """yt (PAIRS,128,6,512) f32 -> y (BL,245,768) f32."""
    arr = (
        np.asarray(yt, np.float32)
        .reshape(PAIRS, 128, 6, 2, NP)
        .transpose(0, 3, 4, 2, 1)
        .reshape(BL // 2 * 2 and PAIRS * 2, NP, D)[:, :N, :]
    )
    return np.ascontiguousarray(arr)


# ------------------------------------------------------------- bass program

def _patch_tile_drain():
    """This walrus build only accepts one sync-wait on a Drain instruction;
    spread the Tile tail-drain waits over several drains."""
    import concourse.tile as tile
    from concourse import mybir
    from concourse.vector_clock import ScopedClock

    if getattr(tile.TileContext, "_drain_patched", False):
        return

    def _drain_and_barrier(self, tick_clock, wait_clock):
        drain_inst = self.nc.sync.drain()
        wait_clock.add_sem_waits(
            drain_inst.ins, ScopedClock({None: tick_clock.global_clock})
        )
        si = drain_inst.ins.sync_info
        waits = list(si.on_wait)
        if len(waits) > 1:
            drain_inst.ins.sync_info = mybir.SyncInfo(
                on_wait=waits[:1], on_update=list(si.on_update)
            )
            for i in range(1, len(waits)):
                extra = self.nc.sync.drain()
                extra.ins.sync_info = mybir.SyncInfo(
                    on_wait=waits[i : i + 1], on_update=[]
                )
        self.nc.all_engine_barrier()
        assert self.sems is not None
        popped = self.nc._tile_sem_poison_stack.pop()
        assert popped is self._sem_poison
        self.nc.clear_and_free_semaphores(list(self.sems.allocated().values()))
        self.nc.all_engine_barrier()

    tile.TileContext._drain_and_barrier = _drain_and_barrier
    tile.TileContext._drain_patched = True


def _build_bass():
    import concourse.bass as bass
    import concourse.tile as tile
    from concourse import bacc
    from concourse import mybir

    _patch_tile_drain()

    bf = mybir.dt.bfloat16
    f32 = mybir.dt.float32
    Exp = mybir.ActivationFunctionType.Exp
    Ident = mybir.ActivationFunctionType.Identity

    nc = bacc.Bacc()
    xt_d = nc.dram_tensor("xt", [PAIRS, 128, 6, 2 * NP], bf, kind="ExternalInput")
    wqk_d = nc.dram_tensor("wqk", [128, 6, 2 * D], bf, kind="ExternalInput")
    wv_d = nc.dram_tensor("wv", [128, 6, D], bf, kind="ExternalInput")
    pw_d = nc.dram_tensor("pw", [128, 6, D], bf, kind="ExternalInput")
    pb_d = nc.dram_tensor("pb", [128, 6], f32, kind="ExternalInput")
    ebt_d = nc.dram_tensor("ebt", [2, 128, H * NP], bf, kind="ExternalInput")
    lhot_d = nc.dram_tensor("lhot", [128, H * H], bf, kind="ExternalInput")
    yt_d = nc.dram_tensor("yt", [PAIRS, 128, 6, 2 * NP], f32, kind="ExternalOutput")

    with tile.TileContext(nc) as tc:
        with (
            tc.tile_pool(name="const", bufs=1) as constp,
            tc.tile_pool(name="px", bufs=2) as px,
            tc.tile_pool(name="pqk", bufs=2) as pqk,
            tc.tile_pool(name="pv", bufs=2) as pv,
            tc.tile_pool(name="per", bufs=3) as per,
            tc.tile_pool(name="pet", bufs=4) as pet,
            tc.tile_pool(name="prc", bufs=2) as prc,
            tc.tile_pool(name="prb", bufs=2) as prb,
            tc.tile_pool(name="pot", bufs=2) as pot,
            tc.tile_pool(name="py", bufs=2) as pysb,
            tc.tile_pool(name="pdram", bufs=2, space="DRAM") as pdram,
            tc.tile_pool(name="ppq", bufs=2, space="PSUM") as ppq,
            tc.tile_pool(name="pst", bufs=3, space="PSUM") as pst,
            tc.tile_pool(name="ppo", bufs=3, space="PSUM") as ppo,
        ):
            wqk_sb = constp.tile([128, 6, 2 * D], bf)
            nc.sync.dma_start(wqk_sb[:], wqk_d[:])
            wv_sb = constp.tile([128, 6, D], bf)
            nc.sync.dma_start(wv_sb[:], wv_d[:])
            pw_sb = constp.tile([128, 6, D], bf)
            nc.sync.dma_start(pw_sb[:], pw_d[:])
            pb_sb = constp.tile([128, 6], f32)
            nc.sync.dma_start(pb_sb[:], pb_d[:])
            ebt_sb = [constp.tile([128, H * NP], bf, name=f"ebt{j}") for j in range(2)]
            for j in range(2):
                nc.sync.dma_start(ebt_sb[j][:], ebt_d[j])
            lhot_sb = constp.tile([128, H * H], bf)
            nc.sync.dma_start(lhot_sb[:], lhot_d[:])

            for p_ in range(REPS * PAIRS):
                p = p_ % PAIRS
                xt = px.tile([128, 6, 2 * NP], bf, tag="xt")
                nc.sync.dma_start(xt[:], xt_d[p])

                # ---- qk projection, transposed output [feat, tok]
                qk_sb = pqk.tile([128, H, 2 * NP], bf, tag="qk")
                for m in range(12):
                    ps = ppq.tile([128, 2 * NP], f32, tag="qv")
                    for kc in range(6):
                        nc.tensor.matmul(
                            ps[:],
                            lhsT=wqk_sb[:, kc, m * 128 : (m + 1) * 128],
                            rhs=xt[:, kc, :],
                            start=(kc == 0),
                            stop=(kc == 5),
                        )
                    nc.scalar.copy(out=qk_sb[:, m, :], in_=ps[:])

                if STAGE < 2:
                    y_sb = pysb.tile([128, 6, 2 * NP], f32, tag="y")
                    nc.vector.tensor_copy(y_sb[:], qk_sb[:, :6, :])
                    nc.sync.dma_start(yt_d[p], y_sb[:])
                    continue

                # ---- v projection, normal layout [tok, feat]
                v_sb = pv.tile([128, 4, D], bf, tag="v")
                for mc in range(4):
                    for nh in range(2):
                        ps = ppq.tile([128, 2 * NP], f32, tag="qv")
                        psl_ = ps[:, : D // 2]
                        for kc in range(6):
                            nc.tensor.matmul(
                                psl_,
                                lhsT=xt[:, kc, mc * 128 : (mc + 1) * 128],
                                rhs=wv_sb[:, kc, nh * (D // 2) : (nh + 1) * (D // 2)],
                                start=(kc == 0),
                                stop=(kc == 5),
                            )
                        nc.scalar.copy(
                            out=v_sb[:, mc, nh * (D // 2) : (nh + 1) * (D // 2)],
                            in_=psl_,
                        )

                if STAGE < 3:
                    y_sb = pysb.tile([128, 6, 2 * NP], f32, tag="y")
                    nc.vector.tensor_copy(y_sb[:, :4, :], v_sb[:, :, :512])
                    nc.sync.dma_start(yt_d[p], y_sb[:])
                    continue

                ot = pot.tile([128, 6, 2 * NP], bf, tag="ot")

                for it in range(2):
                    tb = it * NP
                    # ---- scores (transposed) + exp + bias-multiply
                    et = [
                        pet.tile([128, H * NP], bf, tag="et", name=f"et{jc}")
                        for jc in range(2)
                    ]
                    for jc in range(2):
                        jsz = JSZ[jc]
                        jst = tb + jc * 128
                        for h in range(H):
                            ps = pst.tile([128, NP], f32, tag="st")
                            fc, ko = h // 2, (h % 2) * 64
                            nc.tensor.matmul(
                                ps[0:jsz, :],
                                lhsT=qk_sb[ko : ko + 64, 6 + fc, jst : jst + jsz],
                                rhs=qk_sb[ko : ko + 64, fc, tb : tb + NP],
                                start=True,
                                stop=True,
                            )
                            er = per.tile([128, NP], bf, tag="er")
                            nc.scalar.activation(er[0:jsz, :], ps[0:jsz, :], func=Exp)
                            nc.vector.tensor_mul(
                                et[jc][0:jsz, h * NP : (h + 1) * NP],
                                er[0:jsz, :],
                                ebt_sb[jc][0:jsz, h * NP : (h + 1) * NP],
                            )

                    if STAGE < 4:
                        nc.vector.tensor_copy(ot[:, 0, tb : tb + NP], et[0][:, :NP])
                        continue

                    # ---- row sums l[h, i] via one-hot matmuls
                    psl = ppo.tile([H, NP], f32, tag="ol", name="psl")
                    nmm = 0
                    for jc in range(2):
                        jsz = JSZ[jc]
                        for h in range(H):
                            nc.tensor.matmul(
                                psl[:],
                                lhsT=lhot_sb[0:jsz, h * H : (h + 1) * H],
                                rhs=et[jc][0:jsz, h * NP : (h + 1) * NP],
                                start=(nmm == 0),
                                stop=(nmm == 2 * H - 1),
                            )
                            nmm += 1
                    rcp = prc.tile([H, NP], bf, tag="rcp")
                    with nc.allow_low_precision(reason="bf16 1/l"):
                        nc.vector.reciprocal(rcp[:], psl[:])
                    rdr = pdram.tile([H, NP], bf, tag="rd")
                    nc.sync.dma_start(rdr[:], rcp[:])
                    rb = prb.tile([128, 6, NP], bf, tag="rb")
                    for hh in range(2):
                        src = bass.AP(
                            tensor=rdr.tensor,
                            offset=rdr.offset + hh * NP,
                            ap=[[0, 64], [2 * NP, 6], [1, NP]],
                        )
                        nc.sync.dma_start(rb[hh * 64 : (hh + 1) * 64, :, :], src)

                    if STAGE < 5:
                        for g in range(6):
                            nc.vector.tensor_copy(ot[:, g, tb : tb + NP], rb[:, g, :])
                        continue

                    # ---- attn @ v (outT layout) + 1/l normalize into ot
                    for g in range(6):
                        pso = ppo.tile([128, NP], f32, tag="ol", name="pso")
                        for hh in range(2):
                            h = 2 * g + hh
                            for jc in range(2):
                                jsz = JSZ[jc]
                                nc.tensor.matmul(
                                    pso[hh * 64 : (hh + 1) * 64, :],
                                    lhsT=v_sb[0:jsz, it * 2 + jc, h * 64 : (h + 1) * 64],
                                    rhs=et[jc][0:jsz, h * NP : (h + 1) * NP],
                                    start=(jc == 0),
                                    stop=(jc == 1),
                                    tile_position=(0, hh * 64),
                                )
                        nc.vector.tensor_copy(ot[:, g, tb : tb + NP], pso[:])
                    with nc.allow_low_precision(reason="bf16 softmax normalize"):
                        nc.vector.tensor_mul(
                            ot[:, :, tb : tb + NP],
                            ot[:, :, tb : tb + NP],
                            rb[:],
                        )

                if STAGE < 6:
                    y_sb = pysb.tile([128, 6, 2 * NP], f32, tag="y")
                    nc.vector.tensor_copy(y_sb[:], ot[:])
                    nc.sync.dma_start(yt_d[p], y_sb[:])
                    continue

                # ---- output projection + bias, write transposed y
                y_sb = pysb.tile([128, 6, 2 * NP], f32, tag="y")
                for nn_ in range(6):
                    psy = ppo.tile([128, 2 * NP], f32, tag="ol", name="psy")
                    for cc in range(6):
                        nc.tensor.matmul(
                            psy[:],
                            lhsT=pw_sb[:, cc, nn_ * 128 : (nn_ + 1) * 128],
                            rhs=ot[:, cc, :],
                            start=(cc == 0),
                            stop=(cc == 5),
                        )
                    nc.scalar.activation(
                        y_sb[:, nn_, :],
                        psy[:],
                        func=Ident,
                        bias=pb_sb[:, nn_ : nn_ + 1],
                        scale=1.0,
                    )
                nc.sync.dma_start(yt_d[p], y_sb[:])

    nc.compile()
    return nc


# ----------------------------------------------------------------- execution

@functools.cache
def _get_runner():
    """Build the bass program once and return a cached jitted executor.

    Mirrors concourse.bass2jax.run_bass_via_pjrt but caches the jit so
    repeated kernel() calls (and timing loops) do not recompile.
    """
    import jax
    import jax.numpy as jnp
    from jax.sharding import Mesh, PartitionSpec
    from jax.experimental.shard_map import shard_map

    from concourse import mybir
    from concourse import bass2jax

    bass2jax.install_neuronx_cc_hook()
    nc = _build_bass()

    partition_name = (
        nc.partition_id_tensor.name if nc.partition_id_tensor is not None else None
    )
    in_names, out_names, out_avals = [], [], []
    for alloc in nc.m.functions[0].allocations:
        if not isinstance(alloc, mybir.MemoryLocationSet):
            continue
        name = alloc.memorylocations[0].name
        if alloc.kind == "ExternalInput":
            if name != partition_name:
                in_names.append(name)
        elif alloc.kind == "ExternalOutput":
            out_names.append(name)
            out_avals.append(
                jax.core.ShapedArray(
                    tuple(alloc.tensor_shape), mybir.dt.np(alloc.dtype)
                )
            )
    n_params = len(in_names)
    all_in_names = tuple(in_names + out_names)
    if partition_name is not None:
        all_in_names = all_in_names + (partition_name,)

    def _body(*args):
        operands = list(args)
        if partition_name is not None:
            operands.append(bass2jax.partition_id_tensor())
        outs = bass2jax._bass_exec_p.bind(
            *operands,
            out_avals=tuple(out_avals),
            in_names=all_in_names,
            out_names=tuple(out_names),
            lowering_input_output_aliases=(),
            sim_require_finite=True,
            sim_require_nnan=True,
            nc=nc,
        )
        return tuple(outs)

    devices = jax.devices()[:NCORES]
    mesh = Mesh(np.asarray(devices), ("core",))
    n_outs = len(out_names)
    donate = tuple(range(n_params, n_params + n_outs))
    sharded = jax.jit(
        shard_map(
            _body,
            mesh=mesh,
            in_specs=(PartitionSpec("core"),) * (n_params + n_outs),
            out_specs=(PartitionSpec("core"),) * n_outs,
            check_rep=False,
        ),
        donate_argnums=donate,
        keep_unused=True,
    )
    return sharded, in_names, out_names, out_avals


def _run_device(per_core_inputs):
    """per_core_inputs: list (len 8) of dicts name->np array. Returns list of
    dicts name->np array (outputs)."""
    sharded, in_names, out_names, out_avals = _get_runner()
    concat_in = [
        np.concatenate([per_core_inputs[c][nm] for c in range(NCORES)], axis=0)
        for nm in in_names
    ]
    concat_zeros = [
        np.zeros((NCORES * a.shape[0], *a.shape[1:]), a.dtype) for a in out_avals
    ]
    out_arrs = sharded(*concat_in, *concat_zeros)
    res = []
    for c in range(NCORES):
        res.append(
            {
                nm: np.asarray(out_arrs[i]).reshape(NCORES, *out_avals[i].shape)[c]
                for i, nm in enumerate(out_names)
            }
        )
    return res


def kernel(x, qkv_w, proj_w, proj_b, bias_table, rel_index):
    x = np.asarray(x, np.float32)
    w = _prep_weights(qkv_w, proj_w, proj_b, bias_table, rel_index)
    per_core = []
    for c in range(NCORES):
        m = dict(w)
        m["xt"] = _prep_x_core(x[c * BL : (c + 1) * BL])
        per_core.append(m)
    res = _run_device(per_core)
    y = np.concatenate([_decode_y_core(res[c]["yt"]) for c in range(NCORES)], axis=0)
    return y.astype(np.float32)


# ------------------------------------------------- numpy emulation (debug)

def _numpy_sim(x, qkv_w, proj_w, proj_b, bias_table, rel_index, exact=False):
    """Bit-layout-faithful numpy emulation of the device program."""
    bf16 = _bf16()
    cast = (lambda a: a.astype(np.float32)) if exact else (
        lambda a: a.astype(bf16).astype(np.float32)
    )
    w = _prep_weights(qkv_w, proj_w, proj_b, bias_table, rel_index)
    wqk = w["wqk"].astype(np.float32)  # (128, 6, 1536)
    wv = w["wv"].astype(np.float32)
    pw = w["pw"].astype(np.float32)
    pb = w["pb"].astype(np.float32)
    ebt = w["ebt"].astype(np.float32).reshape(2, 128, H, NP)
    x = np.asarray(x, np.float32)
    y_all = np.zeros((B, N, D), np.float32)
    for c in range(NCORES):
        xt = _prep_x_core(x[c * BL : (c + 1) * BL]).astype(np.float32)
        yt = np.zeros((PAIRS, 128, 6, 2 * NP), np.float32)
        for p in range(PAIRS):
            xtp = xt[p]  # (128, 6, 512)
            qk = np.zeros((128, H, 2 * NP), np.float32)
            for m in range(12):
                acc = np.zeros((128, 2 * NP), np.float32)
                for kc in range(6):
                    acc += wqk[:, kc, m * 128 : (m + 1) * 128].T @ xtp[:, kc, :]
                qk[:, m, :] = cast(acc)
            v = np.zeros((128, 4, D), np.float32)
            for mc in range(4):
                acc = np.zeros((128, D), np.float32)
                for kc in range(6):
                    acc += xtp[:, kc, mc * 128 : (mc + 1) * 128].T @ wv[:, kc, :]
                v[:, mc, :] = cast(acc)
            ot = np.zeros((128, 6, 2 * NP), np.float32)
            for it in range(2):
                tb = it * NP
                et = [np.zeros((128, H, NP), np.float32) for _ in range(2)]
                for jc in range(2):
                    jsz = JSZ[jc]
                    jst = tb + jc * 128
                    for h in range(H):
                        fc, ko = h // 2, (h % 2) * 64
                        st = (
                            qk[ko : ko + 64, 6 + fc, jst : jst + jsz].T
                            @ qk[ko : ko + 64, fc, tb : tb + NP]
                        )
                        er = cast(np.exp(st))
                        et[jc][0:jsz, h, :] = cast(er * ebt[jc][0:jsz, h, :])
                l = np.zeros((H, NP), np.float32)
                for jc in range(2):
                    l += et[jc][: JSZ[jc]].sum(axis=0)
                rcp = 1.0 / l
                for g in range(6):
                    for hh in range(2):
                        h = 2 * g + hh
                        acc = np.zeros((64, NP), np.float32)
                        for jc in range(2):
                            jsz = JSZ[jc]
                            acc += (
                                v[0:jsz, it * 2 + jc, h * 64 : (h + 1) * 64].T
                                @ et[jc][0:jsz, h, :]
                            )
                        ot[hh * 64 : (hh + 1) * 64, g, tb : tb + NP] = cast(
                            acc * rcp[h][None, :]
                        )
                for nn_ in range(6):
                    acc = np.zeros((128, 2 * NP), np.float32)
                    for cc in range(6):
                        acc += pw[:, cc, nn_ * 128 : (nn_ + 1) * 128].T @ ot[:, cc, :]
                    yt[p, :, nn_, :] = acc + pb[:, nn_ : nn_ + 1]
        y_all[c * BL : (c + 1) * BL] = _decode_y_core(yt)
    return y_all
